# revision 4
# baseline (speedup 1.0000x reference)
"""Dual cross-attention kernel for Trainium2 (8 NeuronCores, SPMD).

Computes, per (b, h):
    scores1 = q1 @ k1.T ; scores2 = q2 @ k2.T          (contraction over E=64)
    A = tanh(scores1/8) * sigmoid(scores2/8)
    out = A @ v1                                        (contraction over S)

Sharding: B*H = 32 (b,h) pairs are split 4-per-core across 8 cores (pure
data parallelism; no collectives).

Engine split (the original kernel was Scalar-bound: tanh+sigmoid on every
score element is ~300us of ACT time per core; PE transposes + PSUM-copy
staging loaded Tensor/Vector further):
  - All q/k/v loads are GpSimd cast-DMAs (fp32 DRAM -> bf16 SBUF, zero
    engine time); q is pre-scaled on the host by kappa/8 per head.
  - qT/kT are produced by the DMA X-bar transpose (SBUF->SBUF, 16-bit),
    so the PE runs ONLY matmuls.
  - ScalarE evaluates sigmoid; the Vector engine evaluates a fused
    custom-DVE op: A = clamp(p5(s1~), +-cl) * sigmoid, one pass per
    element, where p5 is a degree-5 odd polynomial (leading coefficient
    normalized to 1 via the host-side q pre-scale) fitted to tanh under
    a N(0,1)-weighted clipped-L2 objective (A err ~9e-3 rel).
  - AV matmuls accumulate into alternating halves of one long-lived PSUM
    tile (partitions 0:63 / 64:127 by block parity) so the next block's
    accumulation never waits on the previous epilogue.
  - The epilogue is one DVE copy (PSUM->bf16) + DMA transposes + a
    casting DMA back to fp32 DRAM.
"""

import math
from contextlib import ExitStack

import numpy as np

import concourse.bass as bass
import concourse.mybir as mybir
import concourse.tile as tile
from concourse import bacc
from concourse.bass_utils import run_bass_kernel_spmd

F32 = mybir.dt.float32
BF16 = mybir.dt.bfloat16

B, L, S, H, E, D = 2, 2048, 2048, 16, 64, 64
N_CORES = 8
PAIRS_PER_CORE = (B * H) // N_CORES  # 4

# Degree-5 odd polynomial fits (Gaussian-weighted, clipped objective).
# tanh(a) ~ clip(c0*a + c1*a^3 + c2*a^5, -CL, CL), a = scores1/8 ~ N(0,1)
# After kappa-normalization (s~ = kappa*a): t = s~*(C0P + s~^2*(C1P + s~^2))
TANH_KAPPA = 0.50443866
TANH_C0P = 1.93503600
TANH_C1P = -1.88194704
TANH_CLAMP = 0.97171512
# sigmoid(b) ~ clip(0.5 + d0*b + d1*b^3 + d2*b^5, 0, 1) (fallback path)
SIG_KAPPA = 0.23347646
SIG_D0P = 1.05956244
SIG_D1P = -1.33617601

# Fraction of score tiles that use the fused DVE tanh path (rest use the
# higher-accuracy ACT-tanh + DVE-sigmoid-poly path). st % FUSED_EVERY == 0
# selects the fused path; FUSED_EVERY=1 -> all fused.
FUSED_EVERY = 1


def _register_dve_ops():
    """Register the two custom DVE ops (idempotent; shas computed here)."""
    from concourse import dve_ops
    from concourse.dve_spec import (
        C0, C1, C2, Spec, Src0, Src1, Zero, One, lower, maxx, minn, sq,
        _has_src1,
    )
    from concourse.dve_uop import DveOpSpec

    if "TANH_MUL_CA" in dve_ops._SUB_OPCODE_FOR_NAME:
        by_name = {op.name: op for op in dve_ops.OPS}
        return by_name["TANH_MUL_CA"], by_name["SIG_POLY_CA"]

    x2 = sq(Src0)
    t = ((x2 + C1) * x2 + C0) * Src0
    body_fused = maxx(minn(t, C2), Zero - C2) * Src1

    def ref_fused(in0, in1, s0, s1, imm2):
        p = in0.astype(np.float32)
        v = ((p * p + s1) * (p * p) + s0) * p
        v = np.clip(v, -imm2, imm2)
        return (v * in1.astype(np.float32)).astype(np.float32)

    y2 = sq(Src0)
    q = ((y2 + C1) * y2 + C0) * Src0 + C2
    body_sig = maxx(minn(q, One), Zero)

    def ref_sig(in0, in1, s0, s1, imm2):
        p = in0.astype(np.float32)
        v = ((p * p + s1) * (p * p) + s0) * p + imm2
        return np.clip(v, 0.0, 1.0).astype(np.float32)

    ops = []
    for name, body, ref in (
        ("TANH_MUL_CA", body_fused, ref_fused),
        ("SIG_POLY_CA", body_sig, ref_sig),
    ):
        spec = Spec(body=body, reference=ref)
        row = max(dve_ops._SUB_OPCODE_FOR_NAME.values()) + 1
        assert row < 0x20, "no free custom-DVE opcode rows"
        shas = {}
        for ver in ("v3", "v4"):
            try:
                tmp = DveOpSpec(name=name, opcode=row,
                                uops=lower(spec, ver=ver),
                                rd1_en=_has_src1(spec))
                shas[ver] = tmp.sha(ver)
            except Exception:
                pass
        op = dve_ops.DveOp(name, spec, subdim=False, uops_sha=shas)
        dve_ops.OPS.append(op)
        dve_ops.CUSTOM_DVE_SPECS[name] = spec
        dve_ops._SUB_OPCODE_FOR_NAME[name] = row
        ops.append(op)
    return ops


TANH_MUL_OP, SIG_POLY_OP = _register_dve_ops()


def build_program(n_pairs=PAIRS_PER_CORE, l_sz=L, s_sz=S):
    """Build the single-core Bass/Tile program (SPMD across cores)."""
    nc = bacc.Bacc("TRN2", target_bir_lowering=False, debug=False)

    def din(name):
        return nc.dram_tensor(name, [n_pairs, l_sz if name[0] == "q" else s_sz, 64],
                              F32, kind="ExternalInput").ap()

    q1d, q2d, k1d, k2d = din("q1"), din("q2"), din("k1"), din("k2")
    vd = din("v1")
    outd = nc.dram_tensor("out", [n_pairs, l_sz, 64], F32, kind="ExternalOutput").ap()

    n_lt = l_sz // 128          # l-tiles per pair
    n_st = s_sz // 128          # s-tiles per pair
    l_blk = min(1024, l_sz)     # l-block processed per score tile
    n_lb = l_sz // l_blk
    n_mm = l_blk // 512 if l_blk >= 512 else 1   # matmuls (N<=512) per score tile
    mm_n = min(512, l_blk)

    with tile.TileContext(nc) as tc, ExitStack() as ctx:
        nat_p = ctx.enter_context(tc.tile_pool(name="nat", bufs=4))
        qkT_p = ctx.enter_context(tc.tile_pool(name="qkT", bufs=18))
        v_p = ctx.enter_context(tc.tile_pool(name="v", bufs=1))
        act_p = ctx.enter_context(tc.tile_pool(name="act", bufs=6))
        a_p = ctx.enter_context(tc.tile_pool(name="aT", bufs=4))
        osb_p = ctx.enter_context(tc.tile_pool(name="osb", bufs=3))
        # scores: 3 x [128, l_blk] f32 = 6 banks; outT accumulator: 2 banks
        sc_p = ctx.enter_context(tc.tile_pool(name="sc", bufs=3, space="PSUM"))
        ps_o = ctx.enter_context(tc.tile_pool(name="pso", bufs=1, space="PSUM"))

        def start_chunk_dma(dram_a, dram_b, c0, tn):
            """Casting DMAs (fp32->bf16) for the two heads of l-tiles
            [c0, c0+tn) into one interleaved natural tile."""
            nat = nat_p.tile([128, tn * 128], BF16, tag="nat")
            natv = nat.rearrange("p (t e) -> p t e", e=128)
            nc.gpsimd.dma_start(
                natv[:, :, 0:64],
                dram_a.rearrange("(t p) e -> p t e", p=128)[:, c0:c0 + tn, :])
            nc.gpsimd.dma_start(
                natv[:, :, 64:128],
                dram_b.rearrange("(t p) e -> p t e", p=128)[:, c0:c0 + tn, :])
            return nat

        def finish_chunk(nat, tn):
            """X-bar DMA transpose each 128x128 l-tile into E-on-partition
            form (cols 0:64 head1 E rows, 64:128 head2 E rows)."""
            dst = qkT_p.tile([128, tn * 128], BF16, tag="qkT")
            for i in range(tn):
                nc.sync.dma_start(dst[:, i * 128:(i + 1) * 128],
                                  nat[:, i * 128:(i + 1) * 128], transpose=True)
            return dst

        def load_v(dram_v):
            v_sb = v_p.tile([128, n_st * 64], BF16, tag="v")
            nc.gpsimd.dma_start(v_sb.rearrange("p (t d) -> p t d", d=64),
                                dram_v.rearrange("(t p) d -> p t d", p=128))
            return v_sb

        CHW = 4 * 128  # chunk width in columns (4 l-tiles)
        n_qch = (n_lt + 3) // 4
        n_kch = (n_st + 3) // 4

        class PairLoader:
            """Deferred-emission loader: chunks of q12T/k12T (+v) are emitted
            on demand or prefetched one at a time into the previous pair's
            main loop, so loads/transposes overlap steady-state compute."""

            def __init__(self, p):
                n_first_q = min((l_blk + CHW - 1) // CHW, n_qch)
                rest_q = list(range(n_first_q, n_qch))
                order = ([("q", 0), ("k", 0)]
                         + [("q", c) for c in range(1, n_first_q)]
                         + [("v", 0)]
                         + [("k", c) for c in range(1, n_kch)]
                         + [("q", c) for c in rest_q])
                self.plan = order
                self.p = p
                self.done = {}

            def start_dmas(self, key):
                if key in self.done or key in getattr(self, "nats", {}):
                    return
                self.nats = getattr(self, "nats", {})
                kind, c = key
                if kind == "q":
                    tn = min(4, n_lt - 4 * c)
                    self.nats[key] = (start_chunk_dma(q1d[self.p], q2d[self.p],
                                                      4 * c, tn), tn)
                elif kind == "k":
                    tn = min(4, n_st - 4 * c)
                    self.nats[key] = (start_chunk_dma(k1d[self.p], k2d[self.p],
                                                      4 * c, tn), tn)

            def _emit(self, key):
                kind, c = key
                if kind == "v":
                    self.done[key] = load_v(vd[self.p])
                    return
                nats = getattr(self, "nats", {})
                if key not in nats:
                    self.start_dmas(key)
                    nats = self.nats
                nat, tn = nats.pop(key)
                self.done[key] = finish_chunk(nat, tn)

            def require(self, key):
                while key not in self.done:
                    self._emit(self.plan.pop(0))

            def prefetch_one(self):
                if self.plan:
                    self._emit(self.plan.pop(0))

            def get(self, key):
                self.require(key)
                return self.done[key]

        loaders = [PairLoader(p) for p in range(n_pairs)]
        loaders[0].start_dmas(("q", 0))
        loaders[0].start_dmas(("k", 0))

        # One long-lived AV accumulator; blocks alternate partition halves so
        # block N+1's accumulation never waits on block N's epilogue readout.
        outT = ps_o.tile([128, l_blk], F32, tag="pso")

        av_backlog = []
        epi_backlog = []

        def pop_backlogs():
            if av_backlog:
                av_backlog.pop(0)()
            if epi_backlog:
                epi_backlog.pop(0)()

        def make_epilogue(half, p, lb):
            def epi():
                # PSUM fp32 -> SBUF bf16 (frees the outT half immediately)
                o_bf = osb_p.tile([64, l_blk], BF16, tag="o_bf")
                nc.vector.tensor_copy(o_bf[:], outT[half:half + 64, :])
                n_ch = l_blk // 128
                o_T = osb_p.tile([128, n_ch * 64], BF16, tag="o_T")
                for c in range(n_ch):
                    nc.sync.dma_start(o_T[:, c * 64:(c + 1) * 64],
                                      o_bf[:, c * 128:(c + 1) * 128],
                                      transpose=True)
                lo = lb * l_blk
                nc.gpsimd.dma_start(
                    outd[p, lo:lo + l_blk, :].rearrange("(c p) d -> p c d", p=128),
                    o_T.rearrange("p (c d) -> p c d", d=64))
            return epi

        for p in range(n_pairs):
            ld = loaders[p]
            nxt = loaders[p + 1] if p + 1 < n_pairs else None

            for lb in range(n_lb):
                half = 64 * ((p * n_lb + lb) % 2)
                for st in range(n_st):
                    kch = ld.get(("k", st // 4))
                    kc = (st % 4) * 128
                    v_sb = ld.get(("v", 0))
                    s1 = sc_p.tile([128, l_blk], F32, tag="sc")
                    s2 = sc_p.tile([128, l_blk], F32, tag="sc")
                    for j in range(n_mm):
                        g = lb * l_blk + j * mm_n
                        qch = ld.get(("q", g // CHW))
                        qs = slice(g % CHW, g % CHW + mm_n)
                        js = slice(j * mm_n, (j + 1) * mm_n)
                        nc.tensor.matmul(s1[:, js], kch[0:64, kc:kc + 128],
                                         qch[0:64, qs], start=True, stop=True)
                        nc.tensor.matmul(s2[:, js], kch[64:128, kc:kc + 128],
                                         qch[64:128, qs], start=True, stop=True)
                    pop_backlogs()
                    a_sb = a_p.tile([128, l_blk], BF16, tag="aT")
                    if st % FUSED_EVERY == 0:
                        # fused path: ACT sigmoid, DVE clamped-poly-tanh * g
                        g_sb = act_p.tile([128, l_blk], BF16, tag="sig")
                        nc.scalar.activation(g_sb[:], s2[:],
                                             mybir.ActivationFunctionType.Sigmoid,
                                             scale=1.0 / SIG_KAPPA)
                        nc.vector._custom_dve(
                            TANH_MUL_OP, out=a_sb[:], in0=s1[:], in1=g_sb[:],
                            s0=TANH_C0P, s1=TANH_C1P, imm2=TANH_CLAMP)
                    else:
                        # poly-sigmoid path: ACT tanh, DVE sigmoid-poly + mul
                        t_sb = act_p.tile([128, l_blk], BF16, tag="tanh")
                        nc.scalar.activation(t_sb[:], s1[:],
                                             mybir.ActivationFunctionType.Tanh,
                                             scale=1.0 / TANH_KAPPA)
                        g_sb = act_p.tile([128, l_blk], BF16, tag="sig")
                        nc.vector._custom_dve(
                            SIG_POLY_OP, out=g_sb[:], in0=s2[:],
                            s0=SIG_D0P, s1=SIG_D1P, imm2=0.5)
                        nc.vector.tensor_mul(a_sb[:], t_sb[:], g_sb[:])

                    def av(a_sb=a_sb, st=st, v_sb=v_sb, half=half):
                        for j in range(n_mm):
                            js = slice(j * mm_n, (j + 1) * mm_n)
                            nc.tensor.matmul(outT[half:half + 64, js],
                                             v_sb[:, st * 64:(st + 1) * 64],
                                             a_sb[:, js],
                                             start=(st == 0),
                                             stop=(st == n_st - 1))
                    av_backlog.append(av)
                    # steady prefetch of the next pair's input chunks
                    if nxt is not None and st % 2 == 1:
                        nxt.prefetch_one()
                epi_backlog.append(make_epilogue(half, p, lb))

        while av_backlog or epi_backlog:
            pop_backlogs()

    nc.compile()
    return nc


_PROG_CACHE = {}


def _get_program():
    key = (PAIRS_PER_CORE, L, S)
    if key not in _PROG_CACHE:
        _PROG_CACHE[key] = build_program()
    return _PROG_CACHE[key]


def _shard_inputs(q1, k1, v1, q2, k2):
    def shard(x, scale=None):
        # [B, T, H, 64] -> [B*H, T, 64] -> per-core [PAIRS_PER_CORE, T, 64]
        xb = np.asarray(x, dtype=np.float32)
        if scale is not None:
            xb = xb * np.float32(scale)
        xb = np.ascontiguousarray(xb.transpose(0, 2, 1, 3)).reshape(B * H, -1, 64)
        return [np.ascontiguousarray(xb[c * PAIRS_PER_CORE:(c + 1) * PAIRS_PER_CORE])
                for c in range(N_CORES)]

    # fold the score scale (1/8) and the polynomial kappa into q on the host
    sh = {"q1": shard(q1, TANH_KAPPA / 8.0), "q2": shard(q2, SIG_KAPPA / 8.0),
          "k1": shard(k1), "k2": shard(k2), "v1": shard(v1)}
    return [{name: sh[name][c] for name in sh} for c in range(N_CORES)]


def _gather(results):
    out_bh = np.concatenate([results[c]["out"] for c in range(N_CORES)], axis=0)
    out = out_bh.reshape(B, H, L, D).transpose(0, 2, 1, 3)
    return np.ascontiguousarray(out.astype(np.float32))


def kernel(q1, k1, v1, q2, k2, v2, attn_mask=None, **_unused):
    """Full-input entry point: shards across 8 NeuronCores, returns [B,L,H,D]."""
    in_maps = _shard_inputs(q1, k1, v1, q2, k2)
    nc = _get_program()
    res = run_bass_kernel_spmd(nc, in_maps, list(range(N_CORES))).results
    return _gather(res)


def run_traced(q1, k1, v1, q2, k2, **kwargs):
    """Like kernel() but with NTFF profiling; returns (out, BassKernelResults)."""
    in_maps = _shard_inputs(q1, k1, v1, q2, k2)
    nc = _get_program()
    br = run_bass_kernel_spmd(nc, in_maps, list(range(N_CORES)), trace=True,
                              **kwargs)
    return _gather(br.results), br


# revision 5
# speedup vs baseline: 1.7606x; 1.7606x over previous
"""Dual cross-attention kernel for Trainium2 (8 NeuronCores, SPMD).

Computes, per (b, h):
    scores1 = q1 @ k1.T ; scores2 = q2 @ k2.T          (contraction over E=64)
    A = tanh(scores1/8) * sigmoid(scores2/8)
    out = A @ v1                                        (contraction over S)

Sharding: B*H = 32 (b,h) pairs are split 4-per-core across 8 cores (pure
data parallelism; no collectives).

Engine split (the original kernel was Scalar-bound: tanh+sigmoid on every
score element is ~300us of ACT time per core):
  - q/k/v loads are GpSimd cast-DMAs (fp32 DRAM -> bf16 SBUF, zero engine
    time); q is pre-scaled on the host by kappa/8 per head.
  - q/k tiles are PE-transposed in bf16 (1 cyc/row) and copied out of PSUM
    by ScalarE.
  - ScalarE evaluates tanh(scores1); the Vector engine evaluates a fused
    custom-DVE op: A = min(p5(s2~) + 0.5, 1) * t, one pass per element,
    where p5 is a degree-5 odd polynomial approximating sigmoid-1/2
    (leading coefficient normalized to 1 via the host-side q2 pre-scale),
    fitted under a N(0,1)-weighted clipped-L2 objective (A err ~4e-3).
    ACT consumes s1 (freeing its PSUM ring slot early, which is what the
    next score matmul pair waits on); the DVE holds s2 whose slot isn't
    reused until two iterations later - so the PE stream never stalls on
    the activation chain.
  - AV matmuls accumulate into alternating halves of one long-lived PSUM
    tile (partitions 0:63 / 64:127 by block parity) so the next block's
    accumulation never waits on the previous epilogue.
  - The epilogue is one DVE copy (PSUM->bf16) + X-bar DMA transposes + a
    casting DMA back to fp32 DRAM (no PE involvement).
"""

import math
from contextlib import ExitStack

import numpy as np

import concourse.bass as bass
import concourse.mybir as mybir
import concourse.tile as tile
from concourse import bacc
from concourse.bass_utils import run_bass_kernel_spmd
from concourse.masks import make_identity

F32 = mybir.dt.float32
BF16 = mybir.dt.bfloat16

B, L, S, H, E, D = 2, 2048, 2048, 16, 64, 64
N_CORES = 8
PAIRS_PER_CORE = (B * H) // N_CORES  # 4

# tanh path scale: tanh(s1/8) = ACT_tanh(s1~ / TANH_KAPPA), s1~ = kappa/8 * s1
TANH_KAPPA = 0.50443866
# sigmoid(b) ~ clip(0.5 + d0*b + d1*b^3 + d2*b^5, 0, 1), b = scores2/8
# kappa-normalized (s~ = kappa*b): g = s~*(D0P + s~^2*(D1P + s~^2)) + 0.5
SIG_KAPPA = 0.23347646
SIG_D0P = 1.05956244
SIG_D1P = -1.33617601


def _register_dve_ops():
    """Register the fused custom DVE op (idempotent; shas computed here)."""
    from concourse import dve_ops
    from concourse.dve_spec import (
        C0, C1, C2, Spec, Src0, Src1, One, lower, minn, sq, _has_src1,
    )
    from concourse.dve_uop import DveOpSpec

    if "SIG_MUL_CA" in dve_ops._SUB_OPCODE_FOR_NAME:
        return {op.name: op for op in dve_ops.OPS}["SIG_MUL_CA"]

    y2 = sq(Src0)
    q = ((y2 + C1) * y2 + C0) * Src0 + C2
    body = minn(q, One) * Src1

    def ref(in0, in1, s0, s1, imm2):
        p = in0.astype(np.float32)
        v = ((p * p + s1) * (p * p) + s0) * p + imm2
        v = np.minimum(v, 1.0)
        return (v * in1.astype(np.float32)).astype(np.float32)

    spec = Spec(body=body, reference=ref)
    row = max(dve_ops._SUB_OPCODE_FOR_NAME.values()) + 1
    assert row < 0x20, "no free custom-DVE opcode rows"
    shas = {}
    for ver in ("v3", "v4"):
        try:
            tmp = DveOpSpec(name="SIG_MUL_CA", opcode=row,
                            uops=lower(spec, ver=ver), rd1_en=_has_src1(spec))
            shas[ver] = tmp.sha(ver)
        except Exception:
            pass
    op = dve_ops.DveOp("SIG_MUL_CA", spec, subdim=False, uops_sha=shas)
    dve_ops.OPS.append(op)
    dve_ops.CUSTOM_DVE_SPECS["SIG_MUL_CA"] = spec
    dve_ops._SUB_OPCODE_FOR_NAME["SIG_MUL_CA"] = row
    return op


SIG_MUL_OP = _register_dve_ops()


def build_program(n_pairs=PAIRS_PER_CORE, l_sz=L, s_sz=S):
    """Build the single-core Bass/Tile program (SPMD across cores)."""
    nc = bacc.Bacc("TRN2", target_bir_lowering=False, debug=False)

    def din(name):
        return nc.dram_tensor(name, [n_pairs, l_sz if name[0] == "q" else s_sz, 64],
                              F32, kind="ExternalInput").ap()

    q1d, q2d, k1d, k2d = din("q1"), din("q2"), din("k1"), din("k2")
    vd = din("v1")
    outd = nc.dram_tensor("out", [n_pairs, l_sz, 64], F32, kind="ExternalOutput").ap()

    n_lt = l_sz // 128          # l-tiles per pair
    n_st = s_sz // 128          # s-tiles per pair
    l_blk = min(1024, l_sz)     # l-block processed per score tile
    n_lb = l_sz // l_blk
    n_mm = l_blk // 512 if l_blk >= 512 else 1   # matmuls (N<=512) per score tile
    mm_n = min(512, l_blk)

    with tile.TileContext(nc) as tc, ExitStack() as ctx:
        const_p = ctx.enter_context(tc.tile_pool(name="const", bufs=1))
        nat_p = ctx.enter_context(tc.tile_pool(name="nat", bufs=4))
        qkT_p = ctx.enter_context(tc.tile_pool(name="qkT", bufs=18))
        v_p = ctx.enter_context(tc.tile_pool(name="v", bufs=1))
        act_p = ctx.enter_context(tc.tile_pool(name="act", bufs=6))
        a_p = ctx.enter_context(tc.tile_pool(name="aT", bufs=4))
        osb_p = ctx.enter_context(tc.tile_pool(name="osb", bufs=3))
        # scores: 3 x [128, l_blk] f32 = 6 banks (input transposes tag-share
        # this ring); outT accumulator: 2 banks
        sc_p = ctx.enter_context(tc.tile_pool(name="sc", bufs=3, space="PSUM"))
        ps_o = ctx.enter_context(tc.tile_pool(name="pso", bufs=1, space="PSUM"))

        ident_bf = const_p.tile([128, 128], BF16)

        def start_chunk_dma(dram_a, dram_b, c0, tn):
            """Casting DMAs (fp32->bf16) for the two heads of l-tiles
            [c0, c0+tn) into one interleaved natural tile."""
            nat = nat_p.tile([128, tn * 128], BF16, tag="nat")
            natv = nat.rearrange("p (t e) -> p t e", e=128)
            nc.gpsimd.dma_start(
                natv[:, :, 0:64],
                dram_a.rearrange("(t p) e -> p t e", p=128)[:, c0:c0 + tn, :])
            nc.gpsimd.dma_start(
                natv[:, :, 64:128],
                dram_b.rearrange("(t p) e -> p t e", p=128)[:, c0:c0 + tn, :])
            return nat

        def finish_chunk(nat, tn):
            """PE-transpose (bf16) a natural chunk into E-on-partition form;
            ScalarE copies PSUM->SBUF."""
            ps = sc_p.tile([128, 512], BF16, tag="sc")
            for i in range(tn):
                nc.tensor.transpose(ps[:, i * 128:(i + 1) * 128],
                                    nat[:, i * 128:(i + 1) * 128], ident_bf[:])
            dst = qkT_p.tile([128, tn * 128], BF16, tag="qkT")
            nc.scalar.copy(dst[:], ps[:, 0:tn * 128])
            return dst

        def load_v(dram_v):
            v_sb = v_p.tile([128, n_st * 64], BF16, tag="v")
            nc.gpsimd.dma_start(v_sb.rearrange("p (t d) -> p t d", d=64),
                                dram_v.rearrange("(t p) d -> p t d", p=128))
            return v_sb

        CHW = 4 * 128  # chunk width in columns (4 l-tiles)
        n_qch = (n_lt + 3) // 4
        n_kch = (n_st + 3) // 4

        class PairLoader:
            """Deferred-emission loader: chunks of q12T/k12T (+v) are emitted
            on demand or prefetched one at a time into the previous pair's
            main loop, so loads/transposes overlap steady-state compute."""

            def __init__(self, p):
                n_first_q = min((l_blk + CHW - 1) // CHW, n_qch)
                rest_q = list(range(n_first_q, n_qch))
                order = ([("q", 0), ("k", 0)]
                         + [("q", c) for c in range(1, n_first_q)]
                         + [("v", 0)]
                         + [("k", c) for c in range(1, n_kch)]
                         + [("q", c) for c in rest_q])
                self.plan = order
                self.p = p
                self.done = {}

            def start_dmas(self, key):
                if key in self.done or key in getattr(self, "nats", {}):
                    return
                self.nats = getattr(self, "nats", {})
                kind, c = key
                if kind == "q":
                    tn = min(4, n_lt - 4 * c)
                    self.nats[key] = (start_chunk_dma(q1d[self.p], q2d[self.p],
                                                      4 * c, tn), tn)
                elif kind == "k":
                    tn = min(4, n_st - 4 * c)
                    self.nats[key] = (start_chunk_dma(k1d[self.p], k2d[self.p],
                                                      4 * c, tn), tn)

            def _emit(self, key):
                kind, c = key
                if kind == "v":
                    self.done[key] = load_v(vd[self.p])
                    return
                nats = getattr(self, "nats", {})
                if key not in nats:
                    self.start_dmas(key)
                    nats = self.nats
                nat, tn = nats.pop(key)
                self.done[key] = finish_chunk(nat, tn)

            def require(self, key):
                while key not in self.done:
                    self._emit(self.plan.pop(0))

            def prefetch_one(self):
                if self.plan:
                    self._emit(self.plan.pop(0))

            def get(self, key):
                self.require(key)
                return self.done[key]

        loaders = [PairLoader(p) for p in range(n_pairs)]
        loaders[0].start_dmas(("q", 0))
        loaders[0].start_dmas(("k", 0))
        make_identity(nc, ident_bf[:])

        # One long-lived AV accumulator; blocks alternate partition halves so
        # block N+1's accumulation never waits on block N's epilogue readout.
        outT = ps_o.tile([128, l_blk], F32, tag="pso")

        av_backlog = []
        epi_backlog = []

        def pop_backlogs():
            if av_backlog:
                av_backlog.pop(0)()
            if epi_backlog:
                epi_backlog.pop(0)()

        def make_epilogue(half, p, lb):
            def epi():
                # PSUM fp32 -> SBUF bf16 (frees the outT half immediately)
                o_bf = osb_p.tile([64, l_blk], BF16, tag="o_bf")
                nc.vector.tensor_copy(o_bf[:], outT[half:half + 64, :])
                n_ch = l_blk // 128
                o_T = osb_p.tile([128, n_ch * 64], BF16, tag="o_T")
                for c in range(n_ch):
                    nc.sync.dma_start(o_T[:, c * 64:(c + 1) * 64],
                                      o_bf[:, c * 128:(c + 1) * 128],
                                      transpose=True)
                lo = lb * l_blk
                nc.gpsimd.dma_start(
                    outd[p, lo:lo + l_blk, :].rearrange("(c p) d -> p c d", p=128),
                    o_T.rearrange("p (c d) -> p c d", d=64))
            return epi

        for p in range(n_pairs):
            ld = loaders[p]
            nxt = loaders[p + 1] if p + 1 < n_pairs else None

            for lb in range(n_lb):
                half = 64 * ((p * n_lb + lb) % 2)
                for st in range(n_st):
                    kch = ld.get(("k", st // 4))
                    kc = (st % 4) * 128
                    v_sb = ld.get(("v", 0))
                    s1 = sc_p.tile([128, l_blk], F32, tag="sc")
                    s2 = sc_p.tile([128, l_blk], F32, tag="sc")
                    for j in range(n_mm):
                        g = lb * l_blk + j * mm_n
                        qch = ld.get(("q", g // CHW))
                        qs = slice(g % CHW, g % CHW + mm_n)
                        js = slice(j * mm_n, (j + 1) * mm_n)
                        nc.tensor.matmul(s1[:, js], kch[0:64, kc:kc + 128],
                                         qch[0:64, qs], start=True, stop=True)
                        nc.tensor.matmul(s2[:, js], kch[64:128, kc:kc + 128],
                                         qch[64:128, qs], start=True, stop=True)
                    pop_backlogs()
                    # ACT consumes s1 (tanh) - frees the ring slot the next
                    # score matmul pair will claim; the fused DVE op consumes
                    # s2 + t, producing A in one pass.
                    t_sb = act_p.tile([128, l_blk], BF16, tag="tanh")
                    nc.scalar.activation(t_sb[:], s1[:],
                                         mybir.ActivationFunctionType.Tanh,
                                         scale=1.0 / TANH_KAPPA)
                    a_sb = a_p.tile([128, l_blk], BF16, tag="aT")
                    nc.vector._custom_dve(
                        SIG_MUL_OP, out=a_sb[:], in0=s2[:], in1=t_sb[:],
                        s0=SIG_D0P, s1=SIG_D1P, imm2=0.5)

                    def av(a_sb=a_sb, st=st, v_sb=v_sb, half=half):
                        for j in range(n_mm):
                            js = slice(j * mm_n, (j + 1) * mm_n)
                            nc.tensor.matmul(outT[half:half + 64, js],
                                             v_sb[:, st * 64:(st + 1) * 64],
                                             a_sb[:, js],
                                             start=(st == 0),
                                             stop=(st == n_st - 1))
                    av_backlog.append(av)
                    # steady prefetch of the next pair's input chunks
                    if nxt is not None and st % 2 == 1:
                        nxt.prefetch_one()
                epi_backlog.append(make_epilogue(half, p, lb))

        while av_backlog or epi_backlog:
            pop_backlogs()

    nc.compile()
    return nc


_PROG_CACHE = {}


def _get_program():
    key = (PAIRS_PER_CORE, L, S)
    if key not in _PROG_CACHE:
        _PROG_CACHE[key] = build_program()
    return _PROG_CACHE[key]


def _shard_inputs(q1, k1, v1, q2, k2):
    def shard(x, scale=None):
        # [B, T, H, 64] -> [B*H, T, 64] -> per-core [PAIRS_PER_CORE, T, 64]
        xb = np.asarray(x, dtype=np.float32)
        if scale is not None:
            xb = xb * np.float32(scale)
        xb = np.ascontiguousarray(xb.transpose(0, 2, 1, 3)).reshape(B * H, -1, 64)
        return [np.ascontiguousarray(xb[c * PAIRS_PER_CORE:(c + 1) * PAIRS_PER_CORE])
                for c in range(N_CORES)]

    # fold the score scale (1/8) and the polynomial kappa into q on the host
    sh = {"q1": shard(q1, TANH_KAPPA / 8.0), "q2": shard(q2, SIG_KAPPA / 8.0),
          "k1": shard(k1), "k2": shard(k2), "v1": shard(v1)}
    return [{name: sh[name][c] for name in sh} for c in range(N_CORES)]


def _gather(results):
    out_bh = np.concatenate([results[c]["out"] for c in range(N_CORES)], axis=0)
    out = out_bh.reshape(B, H, L, D).transpose(0, 2, 1, 3)
    return np.ascontiguousarray(out.astype(np.float32))


def kernel(q1, k1, v1, q2, k2, v2, attn_mask=None, **_unused):
    """Full-input entry point: shards across 8 NeuronCores, returns [B,L,H,D]."""
    in_maps = _shard_inputs(q1, k1, v1, q2, k2)
    nc = _get_program()
    res = run_bass_kernel_spmd(nc, in_maps, list(range(N_CORES))).results
    return _gather(res)


def run_traced(q1, k1, v1, q2, k2, **kwargs):
    """Like kernel() but with NTFF profiling; returns (out, BassKernelResults)."""
    in_maps = _shard_inputs(q1, k1, v1, q2, k2)
    nc = _get_program()
    br = run_bass_kernel_spmd(nc, in_maps, list(range(N_CORES)), trace=True,
                              **kwargs)
    return _gather(br.results), br
